# revision 46
# baseline (speedup 1.0000x reference)
r"""Trainium2 Bass kernel for the triangular-DP "MAA layer" problem.

Reference computes, per frame t (T=1024, D=256, L=T+1 counts):
    q_t = (1-p_t) q_{t-1} + p_t shift(q_{t-1})          (Poisson-binomial DP)
    m_t = p_t a m_sh + (1-p_t) m + p_t b q_sh x_t       ([L, D] state)
    out = sum_i m_T[i, :]                               ([D])

Algebraic restructuring: the whole scan collapses to

    out[d] = sum_t c_t x[t, d],
    c_t    = p_t * I_t,   I_t = int_0^1 prod_{s != t} ((1-p_s) + p_s u) du.

The integrand is a boundary-layer spike at u=1 of width ~1/S, S = sum_s p_s.
Gauss-Legendre on the rescaled interval [1 - 20/S, 1] (host-computed from p)
is accurate to ~4e-3 with only K=6 nodes (tail-cut error e^-20).  With
f[t,k] = 1 + p_t (u_k - 1):

    slog_k = sum_t ln f[t,k]
    c_t    = p_t * sum_k exp(slog_k + ln w_k - ln f[t,k])
    out    = c^T @ x

Device mapping (t on partitions: 8 chunks of 128; k on free dim, K=6),
replicated on all 8 cores (the 8-core collective latency floor exceeds the
whole compute phase, so sharding loses):
  - one [128, 8+K+K] aux input carries pcol + host-pre-broadcast um1/lnw
    (no device-side row broadcasts); both inputs are split across the
    Sync and ScalarE DMA queues so all packet streams land in parallel
  - single big Ln and single big Exp on ScalarE; the Exp table load hides
    behind the DVE chunk-reduce + the slog matmul
  - the slog partition-reduce uses a [128,128] all-ones stationary so the
    result lands pre-broadcast in PSUM; the p_t factor of c_t is folded
    into the x operand host-side
  - Exp -> k-reduce -> bf16 matmuls pipelined in two 4-chunk halves with
    separate clean PSUM accumulation groups; ScalarE drains the low half
    while the high half runs (x is shipped as bf16, which also halves
    the big DMA; rel err ~3e-3 vs the 2e-2 gate)
  - junk matmuls keep the PE busy ~3us so the PE_HAM activity monitor
    un-throttles the clock gate (1.2 -> 2.4 GHz) before the final
    matmuls; the HAM window is free-running so this lands ~50% of runs
"""
import numpy as np

T, D, NCH, P, K = 1024, 256, 8, 128, 6
N_CORES = 8

_CACHE = {}


def _build_program():
    import concourse.bass as bass
    import concourse.bacc as bacc
    import concourse.mybir as mybir
    import concourse.tile as tile

    f32 = mybir.dt.float32
    bf16 = mybir.dt.bfloat16
    A = mybir.AluOpType
    ACT = mybir.ActivationFunctionType

    nc = bacc.Bacc("TRN2", target_bir_lowering=False, debug=False,
                   num_devices=N_CORES)

    AUXW = NCH + 2 * K  # [pcol(8) | um1bc(K) | lnwbc(K)], host-broadcast
    paux_d = nc.dram_tensor("paux", [P, AUXW], f32, kind="ExternalInput")
    xa_d = nc.dram_tensor("xa", [P, NCH * D], bf16, kind="ExternalInput")
    out_d = nc.dram_tensor("out", [1, D], f32, kind="ExternalOutput")

    with tile.TileContext(nc) as tc:
        with (
            tc.tile_pool(name="sb", bufs=1) as sb,
            tc.tile_pool(name="ps", bufs=1, space=bass.MemorySpace.PSUM) as ps,
        ):
            paux = sb.tile([P, AUXW], f32, tag="paux")
            xa = sb.tile([P, NCH * D], bf16, tag="xa")
            # both inputs split across the two engines' DMA queues so all
            # packet streams flow in parallel (halves each landing window)
            nc.sync.dma_start(paux[0:64, :], paux_d[0:64, :])
            nc.scalar.dma_start(paux[64:P, :], paux_d[64:P, :])
            nc.sync.dma_start(xa[0:64, :], xa_d[0:64, :])
            # xa-hi rides GpSimd's queue: its late spin-up is harmless (xa
            # isn't needed until the final matmuls) and it keeps ScalarE's
            # queue short so the Ln table load finishes sooner
            nc.gpsimd.dma_start(xa[64:P, :], xa_d[64:P, :])
            pcol = paux[:, 0:NCH]
            um1bc = paux[:, NCH:NCH + K]
            lnwbc = paux[:, NCH + K:NCH + 2 * K]

            # PE warmup: junk matmuls lift the PE out of its lowest DVFS
            # p-state while the input DMAs land
            jmv = sb.tile([P, 512], bf16, tag="jmv")
            nc.vector.memset(jmv[:], 0.0)
            onesbig = sb.tile([P, P], f32, tag="onesbig")
            nc.gpsimd.memset(onesbig[:], 1.0)
            jps = ps.tile([1, 512], f32, tag="jps")
            for _ in range(6):
                nc.tensor.matmul(jps[:], jmv[:, 0:1], jmv[:],
                                 start=True, stop=True)

            # fm1[t,(c,k)] = pcol[t,c] * um1[k]
            fm1 = sb.tile([P, NCH * K], f32, tag="fm1")
            um1_rep = um1bc.unsqueeze(1).broadcast_to([P, NCH, K])
            p_rep = pcol.unsqueeze(2).broadcast_to([P, NCH, K])
            nc.vector.tensor_tensor(fm1.rearrange("p (c k) -> p c k", c=NCH),
                                    um1_rep, p_rep, op=A.mult)

            slogbc_ps = ps.tile([P, K], f32, tag="slogbc_ps")

            # lf = Ln(fm1 + 1)
            lfbig = sb.tile([P, NCH * K], f32, tag="lfbig")
            nc.scalar.activation(lfbig[:], fm1[:], ACT.Ln, bias=1.0)

            # lfsum[t, k] = sum_c lf[t,(c,k)] : one strided reduce (at K=8
            # the strided read is only 64 elements/partition, so one op
            # beats three halving adds' fixed overheads)
            lfsum = sb.tile([P, K], f32, tag="lfsum")
            nc.vector.tensor_reduce(
                lfsum[:], lfbig.rearrange("p (c k) -> p k c", c=NCH),
                axis=mybir.AxisListType.X, op=A.add)

            # all-ones-stationary partition-reduce lands slog pre-broadcast
            nc.tensor.matmul(slogbc_ps[:], onesbig[:], lfsum[:],
                             start=True, stop=True)

            # junk matmul on lfsum bridges the PE gap after the slog
            # reduce (keeps the HAM busy-run alive)
            jps2 = ps.tile([1, NCH * K], f32, tag="jps2")
            nc.tensor.matmul(jps2[:, 0:K], onesbig[:, 0:1], lfsum[:],
                             start=True, stop=True)

            # w2 = slog + lnw ; arg = w2 - lf ; e = exp(arg)
            w2 = sb.tile([P, K], f32, tag="w2")
            nc.vector.tensor_tensor(w2[:], slogbc_ps[:], lnwbc, op=A.add)
            arg = sb.tile([P, NCH * K], f32, tag="arg")
            HH = NCH // 2
            w2_rep = w2.unsqueeze(1).broadcast_to([P, HH, K])
            for hh in range(2):
                ks2, ke2 = hh * HH * K, (hh + 1) * HH * K
                nc.vector.tensor_tensor(
                    arg[:, ks2:ke2].rearrange("p (c k) -> p c k", c=HH),
                    w2_rep,
                    lfbig[:, ks2:ke2].rearrange("p (c k) -> p c k", c=HH),
                    op=A.subtract)
            # Exp -> k-reduce -> p-mult -> matmuls, pipelined in two 4-chunk
            # halves; all 8 matmuls accumulate into one PSUM tile
            HC = NCH // 2
            # junk matmul on arg extends the PE busy run so the HAM
            # un-throttle (needs ~3.4us sustained) covers the final matmuls
            nc.tensor.matmul(jps2[:], onesbig[:, 0:1], arg[:],
                             start=True, stop=True)
            e = sb.tile([P, NCH * K], f32, tag="e")
            cfinb = sb.tile([P, NCH], bf16, tag="cfinb")
            out_ps0 = ps.tile([1, D], f32, tag="out_ps0")
            out_ps1 = ps.tile([1, D], f32, tag="out_ps1")
            out_ps = [out_ps0, out_ps1]
            for h in range(2):
                cs, ce = h * HC, (h + 1) * HC
                ks, ke = cs * K, ce * K
                nc.scalar.activation(e[:, ks:ke], arg[:, ks:ke], ACT.Exp)
                with nc.allow_low_precision(
                        reason="6-term reduce straight to bf16; feeds a "
                               "bf16 matmul stationary anyway"):
                    nc.vector.tensor_reduce(
                        cfinb[:, cs:ce],
                        e[:, ks:ke].rearrange("p (c k) -> p c k", c=HC),
                        axis=mybir.AxisListType.X, op=A.add)
                if h == 0:
                    # second junk on arg bridges the last PE gap before the
                    # finals (runs back-to-back with the first)
                    nc.tensor.matmul(jps2[:], onesbig[:, 0:1],
                                     arg[:], start=True, stop=True)
                for c in range(cs, ce):
                    nc.tensor.matmul(out_ps[h][:], cfinb[:, c:c + 1],
                                     xa[:, c * D:(c + 1) * D],
                                     start=(c == cs), stop=(c == ce - 1))
            # two clean accumulation groups (no mid-group semaphore waits):
            # ScalarE drains the low half while the high-half matmuls run,
            # then one DVE add (sbuf + single psum read)
            olo_sb = sb.tile([1, D], f32, tag="olosb")
            nc.scalar.activation(olo_sb[:], out_ps[0][:], ACT.Copy)
            out_sb = sb.tile([1, D], f32, tag="outsb")
            nc.vector.tensor_tensor(out_sb[:], olo_sb[:], out_ps[1][:],
                                    op=A.add)
            nc.sync.dma_start(out_d[:], out_sb[:])

    nc.compile()
    return nc


def _make_in_map(p, x):
    import ml_dtypes

    p = np.ascontiguousarray(np.asarray(p, dtype=np.float32)).reshape(T)
    x = np.ascontiguousarray(np.asarray(x, dtype=np.float32)).reshape(T, D)
    S = float(np.sum(np.asarray(p, np.float64)))
    delta = min(1.0, 20.0 / max(S, 1.0))
    nodes, weights = np.polynomial.legendre.leggauss(K)
    u = 1.0 - delta + delta * (nodes + 1.0) * 0.5
    w = weights * delta * 0.5
    paux = np.empty((P, NCH + 2 * K), np.float32)
    paux[:, 0:NCH] = p.reshape(NCH, P).T
    paux[:, NCH:NCH + K] = (u - 1.0).astype(np.float32)[None, :]
    paux[:, NCH + K:NCH + 2 * K] = np.log(w).astype(np.float32)[None, :]
    xp = x * p[:, None]  # fold the p_t factor of c_t into the x operand
    xa = np.ascontiguousarray(
        xp.reshape(NCH, P, D).transpose(1, 0, 2).reshape(P, NCH * D)
    ).astype(ml_dtypes.bfloat16)
    return {"paux": paux, "xa": xa}


def _run(p, x, trace=False, tmpdir=None):
    from concourse.bass_utils import run_bass_kernel_spmd

    if "nc" not in _CACHE:
        _CACHE["nc"] = _build_program()
    nc = _CACHE["nc"]
    in_map = _make_in_map(p, x)
    in_maps = [in_map for _ in range(N_CORES)]
    res = run_bass_kernel_spmd(nc, in_maps, list(range(N_CORES)),
                               trace=trace, tmpdir=tmpdir)
    out = np.asarray(res.results[0]["out"], dtype=np.float32).reshape(D)
    return out, res


def kernel(p, x):
    out, _ = _run(p, x, trace=False)
    return out


# revision 47
# speedup vs baseline: 1.0068x; 1.0068x over previous
r"""Trainium2 Bass kernel for the triangular-DP "MAA layer" problem.

Reference computes, per frame t (T=1024, D=256, L=T+1 counts):
    q_t = (1-p_t) q_{t-1} + p_t shift(q_{t-1})          (Poisson-binomial DP)
    m_t = p_t a m_sh + (1-p_t) m + p_t b q_sh x_t       ([L, D] state)
    out = sum_i m_T[i, :]                               ([D])

Algebraic restructuring: the whole scan collapses to

    out[d] = sum_t c_t x[t, d],
    c_t    = p_t * I_t,   I_t = int_0^1 prod_{s != t} ((1-p_s) + p_s u) du.

The integrand is a boundary-layer spike at u=1 of width ~1/S, S = sum_s p_s.
Gauss-Legendre on the rescaled interval [1 - 20/S, 1] (host-computed from p)
is accurate to ~4e-3 with only K=6 nodes (tail-cut error e^-20).  With
f[t,k] = 1 + p_t (u_k - 1):

    slog_k = sum_t ln f[t,k]
    c_t    = p_t * sum_k exp(slog_k + ln w_k - ln f[t,k])
    out    = c^T @ x

Device mapping (t on partitions: 8 chunks of 128; k on free dim, K=6),
replicated on all 8 cores (the 8-core collective latency floor exceeds the
whole compute phase, so sharding loses):
  - one [128, 8+K+K] aux input carries pcol + host-pre-broadcast um1/lnw
    (no device-side row broadcasts); both inputs are split across the
    Sync and ScalarE DMA queues so all packet streams land in parallel
  - single big Ln and single big Exp on ScalarE; the Exp table load hides
    behind the DVE chunk-reduce + the slog matmul
  - the slog partition-reduce uses a [128,128] all-ones stationary so the
    result lands pre-broadcast in PSUM; the p_t factor of c_t is folded
    into the x operand host-side
  - Exp -> k-reduce -> bf16 matmuls pipelined in two 4-chunk halves with
    separate clean PSUM accumulation groups; ScalarE drains the low half
    while the high half runs (x is shipped as bf16, which also halves
    the big DMA; rel err ~3e-3 vs the 2e-2 gate)
  - junk matmuls keep the PE busy ~3us so the PE_HAM activity monitor
    un-throttles the clock gate (1.2 -> 2.4 GHz) before the final
    matmuls; the HAM window is free-running so this lands ~50% of runs
"""
import numpy as np

T, D, NCH, P, K = 1024, 256, 8, 128, 6
N_CORES = 8

_CACHE = {}


def _build_program():
    import concourse.bass as bass
    import concourse.bacc as bacc
    import concourse.mybir as mybir
    import concourse.tile as tile

    f32 = mybir.dt.float32
    bf16 = mybir.dt.bfloat16
    A = mybir.AluOpType
    ACT = mybir.ActivationFunctionType

    nc = bacc.Bacc("TRN2", target_bir_lowering=False, debug=False,
                   num_devices=N_CORES)

    AUXW = NCH + 2 * K  # [pcol(8) | um1bc(K) | lnwbc(K)], host-broadcast
    paux_d = nc.dram_tensor("paux", [P, AUXW], f32, kind="ExternalInput")
    xa_d = nc.dram_tensor("xa", [P, NCH * D], bf16, kind="ExternalInput")
    out_d = nc.dram_tensor("out", [1, D], f32, kind="ExternalOutput")

    with tile.TileContext(nc) as tc:
        with (
            tc.tile_pool(name="sb", bufs=1) as sb,
            tc.tile_pool(name="ps", bufs=1, space=bass.MemorySpace.PSUM) as ps,
        ):
            paux = sb.tile([P, AUXW], f32, tag="paux")
            xa = sb.tile([P, NCH * D], bf16, tag="xa")
            # both inputs split across the two engines' DMA queues so all
            # packet streams flow in parallel (halves each landing window)
            nc.sync.dma_start(paux[0:64, :], paux_d[0:64, :])
            nc.scalar.dma_start(paux[64:P, :], paux_d[64:P, :])
            nc.sync.dma_start(xa[0:64, :], xa_d[0:64, :])
            nc.scalar.dma_start(xa[64:P, :], xa_d[64:P, :])
            pcol = paux[:, 0:NCH]
            um1bc = paux[:, NCH:NCH + K]
            lnwbc = paux[:, NCH + K:NCH + 2 * K]

            # PE warmup: junk matmuls lift the PE out of its lowest DVFS
            # p-state while the input DMAs land
            jmv = sb.tile([P, 512], bf16, tag="jmv")
            nc.vector.memset(jmv[:], 0.0)
            onesbig = sb.tile([P, P], f32, tag="onesbig")
            nc.gpsimd.memset(onesbig[:], 1.0)
            jps = ps.tile([1, 512], f32, tag="jps")
            for _ in range(6):
                nc.tensor.matmul(jps[:], jmv[:, 0:1], jmv[:],
                                 start=True, stop=True)

            # fm1[t,(c,k)] = pcol[t,c] * um1[k]
            fm1 = sb.tile([P, NCH * K], f32, tag="fm1")
            um1_rep = um1bc.unsqueeze(1).broadcast_to([P, NCH, K])
            p_rep = pcol.unsqueeze(2).broadcast_to([P, NCH, K])
            nc.vector.tensor_tensor(fm1.rearrange("p (c k) -> p c k", c=NCH),
                                    um1_rep, p_rep, op=A.mult)

            slogbc_ps = ps.tile([P, K], f32, tag="slogbc_ps")

            # lf = Ln(fm1 + 1)
            lfbig = sb.tile([P, NCH * K], f32, tag="lfbig")
            nc.scalar.activation(lfbig[:], fm1[:], ACT.Ln, bias=1.0)

            # lfsum[t, k] = sum_c lf[t,(c,k)] : one strided reduce (at K=8
            # the strided read is only 64 elements/partition, so one op
            # beats three halving adds' fixed overheads)
            lfsum = sb.tile([P, K], f32, tag="lfsum")
            nc.vector.tensor_reduce(
                lfsum[:], lfbig.rearrange("p (c k) -> p k c", c=NCH),
                axis=mybir.AxisListType.X, op=A.add)

            # all-ones-stationary partition-reduce lands slog pre-broadcast
            nc.tensor.matmul(slogbc_ps[:], onesbig[:], lfsum[:],
                             start=True, stop=True)

            # junk matmul on lfsum bridges the PE gap after the slog
            # reduce (keeps the HAM busy-run alive)
            jps2 = ps.tile([1, NCH * K], f32, tag="jps2")
            nc.tensor.matmul(jps2[:, 0:K], onesbig[:, 0:1], lfsum[:],
                             start=True, stop=True)

            # w2 = slog + lnw ; arg = w2 - lf ; e = exp(arg)
            w2 = sb.tile([P, K], f32, tag="w2")
            nc.vector.tensor_tensor(w2[:], slogbc_ps[:], lnwbc, op=A.add)
            arg = sb.tile([P, NCH * K], f32, tag="arg")
            HH = NCH // 2
            w2_rep = w2.unsqueeze(1).broadcast_to([P, HH, K])
            for hh in range(2):
                ks2, ke2 = hh * HH * K, (hh + 1) * HH * K
                nc.vector.tensor_tensor(
                    arg[:, ks2:ke2].rearrange("p (c k) -> p c k", c=HH),
                    w2_rep,
                    lfbig[:, ks2:ke2].rearrange("p (c k) -> p c k", c=HH),
                    op=A.subtract)
            # Exp -> k-reduce -> p-mult -> matmuls, pipelined in two 4-chunk
            # halves; all 8 matmuls accumulate into one PSUM tile
            HC = NCH // 2
            # junk matmul on arg extends the PE busy run so the HAM
            # un-throttle (needs ~3.4us sustained) covers the final matmuls
            nc.tensor.matmul(jps2[:], onesbig[:, 0:1], arg[:],
                             start=True, stop=True)
            e = sb.tile([P, NCH * K], f32, tag="e")
            cfinb = sb.tile([P, NCH], bf16, tag="cfinb")
            out_ps0 = ps.tile([1, D], f32, tag="out_ps0")
            out_ps1 = ps.tile([1, D], f32, tag="out_ps1")
            out_ps = [out_ps0, out_ps1]
            for h in range(2):
                cs, ce = h * HC, (h + 1) * HC
                ks, ke = cs * K, ce * K
                nc.scalar.activation(e[:, ks:ke], arg[:, ks:ke], ACT.Exp)
                with nc.allow_low_precision(
                        reason="6-term reduce straight to bf16; feeds a "
                               "bf16 matmul stationary anyway"):
                    nc.vector.tensor_reduce(
                        cfinb[:, cs:ce],
                        e[:, ks:ke].rearrange("p (c k) -> p c k", c=HC),
                        axis=mybir.AxisListType.X, op=A.add)
                if h == 0:
                    # second junk on arg bridges the last PE gap before the
                    # finals (runs back-to-back with the first)
                    nc.tensor.matmul(jps2[:], onesbig[:, 0:1],
                                     arg[:], start=True, stop=True)
                for c in range(cs, ce):
                    nc.tensor.matmul(out_ps[h][:], cfinb[:, c:c + 1],
                                     xa[:, c * D:(c + 1) * D],
                                     start=(c == cs), stop=(c == ce - 1))
            # two clean accumulation groups (no mid-group semaphore waits):
            # ScalarE drains the low half while the high-half matmuls run,
            # then one DVE add (sbuf + single psum read)
            olo_sb = sb.tile([1, D], f32, tag="olosb")
            nc.scalar.activation(olo_sb[:], out_ps[0][:], ACT.Copy)
            out_sb = sb.tile([1, D], f32, tag="outsb")
            nc.vector.tensor_tensor(out_sb[:], olo_sb[:], out_ps[1][:],
                                    op=A.add)
            nc.sync.dma_start(out_d[:], out_sb[:])

    nc.compile()
    return nc


def _make_in_map(p, x):
    import ml_dtypes

    p = np.ascontiguousarray(np.asarray(p, dtype=np.float32)).reshape(T)
    x = np.ascontiguousarray(np.asarray(x, dtype=np.float32)).reshape(T, D)
    S = float(np.sum(np.asarray(p, np.float64)))
    delta = min(1.0, 20.0 / max(S, 1.0))
    nodes, weights = np.polynomial.legendre.leggauss(K)
    u = 1.0 - delta + delta * (nodes + 1.0) * 0.5
    w = weights * delta * 0.5
    paux = np.empty((P, NCH + 2 * K), np.float32)
    paux[:, 0:NCH] = p.reshape(NCH, P).T
    paux[:, NCH:NCH + K] = (u - 1.0).astype(np.float32)[None, :]
    paux[:, NCH + K:NCH + 2 * K] = np.log(w).astype(np.float32)[None, :]
    xp = x * p[:, None]  # fold the p_t factor of c_t into the x operand
    xa = np.ascontiguousarray(
        xp.reshape(NCH, P, D).transpose(1, 0, 2).reshape(P, NCH * D)
    ).astype(ml_dtypes.bfloat16)
    return {"paux": paux, "xa": xa}


def _run(p, x, trace=False, tmpdir=None):
    from concourse.bass_utils import run_bass_kernel_spmd

    if "nc" not in _CACHE:
        _CACHE["nc"] = _build_program()
    nc = _CACHE["nc"]
    in_map = _make_in_map(p, x)
    in_maps = [in_map for _ in range(N_CORES)]
    res = run_bass_kernel_spmd(nc, in_maps, list(range(N_CORES)),
                               trace=trace, tmpdir=tmpdir)
    out = np.asarray(res.results[0]["out"], dtype=np.float32).reshape(D)
    return out, res


def kernel(p, x):
    out, _ = _run(p, x, trace=False)
    return out
